# revision 3
# baseline (speedup 1.0000x reference)
"""Trainium2 Bass kernel for GQA attention (B=4, S=1024, D=4096, HQ=32, HKV=8).

Sharding: 8 cores = 4 batches x 2 head-groups. Each core computes one batch
with 16 q-heads / 4 kv-heads (Wq/Wk/Wv column-sharded, Wo row-sharded). The
two head-group partial outputs per batch are summed on the host (this is the
Wo-row-shard reduction, done host-side instead of an on-device all-reduce),
then transposed (device emits out^T [Dout, S]) and bias bo added.

Device dataflow per core (SPMD, identical graph):
  Phase 1 (QKV): q[s,dq] = xT_chunk.T @ Wq_chunk (bf16, psum accumulate over
    D), evict + bias via DVE; RoPE on DVE in [s, d] layout using host-permuted
    "evens-first" head columns; PE-transpose q,k -> qT,kT [d, s] (bf16).
  Phase 2 (attention, per head): scores_i = qT_i.T @ kT (causal), diag-block
    mask added in psum, exp+rowsum fused on ScalarE (unsafe softmax - no max
    subtraction; scores are O(30) so fp32 exp is safe), normalize on DVE,
    PE-transpose attn -> attnT, av: oT += v_j.T @ attnT_j.
  Phase 3: outT = Wo_chunk.T @ oT (accumulate over dq), DMA out^T.
"""

import math
import os

import numpy as np
import ml_dtypes

import concourse.bass as bass
import concourse.mybir as mybir
import concourse.tile as tile
from concourse import bacc
from concourse.bass_utils import run_bass_kernel_spmd
from concourse.masks import make_causal_mask, make_identity

B, S, D = 4, 1024, 4096
HQ, HKV, HD = 32, 8, 128
NH = 16          # q heads per core
NKV = 4          # kv heads per core
DQ = NH * HD     # 2048
DK = NKV * HD    # 512
NDC = D // 128   # 32 D-chunks
NSC = S // 128   # 8 s-chunks
QK_SCALE = 1.0 / math.sqrt(HD)
MASK_VAL = -1e30

F32 = mybir.dt.float32
BF16 = mybir.dt.bfloat16

_GRAPH_CACHE = {}
LAST_PROFILE = None


def _bcast_like(small_ap, big_ap):
    """Broadcast small_ap (size-1 dims) against big_ap's shape."""
    a, b = bass.broadcast_tensor_aps(big_ap, small_ap)
    return b


def _build_graph():
    nc = bacc.Bacc(debug=False)

    xt_ext = nc.dram_tensor("xt", [NDC, 128, S], BF16, kind="ExternalInput")
    wq_ext = nc.dram_tensor("wq", [NDC, 128, DQ], BF16, kind="ExternalInput")
    wkv_ext = nc.dram_tensor("wkv", [NDC, 128, DK + DK], BF16, kind="ExternalInput")
    wo_ext = nc.dram_tensor("wo", [NH, 128, D], BF16, kind="ExternalInput")
    fcc_ext = nc.dram_tensor("fcc", [128, NSC, 64], F32, kind="ExternalInput")
    fcd_ext = nc.dram_tensor("fcd", [128, NSC, 64], F32, kind="ExternalInput")
    bqkv_ext = nc.dram_tensor("bqkv", [128, DQ + DK + DK], F32, kind="ExternalInput")
    out_ext = nc.dram_tensor("out", [D, S], F32, kind="ExternalOutput")

    with tile.TileContext(nc) as tc:
        with (
            tc.tile_pool(name="const", bufs=1) as cpool,
            tc.tile_pool(name="persist", bufs=1) as ppool,
            tc.tile_pool(name="stat", bufs=2) as spool,
        ):
            maskbias = cpool.tile([128, 128], F32)
            make_causal_mask(nc, maskbias, mask_val=MASK_VAL)
            ident_f = cpool.tile([128, 128], F32)
            make_identity(nc, ident_f)
            ident_b = cpool.tile([128, 128], BF16)
            make_identity(nc, ident_b)
            fcc_sb = cpool.tile([128, NSC, 64], F32)
            nc.sync.dma_start(out=fcc_sb[:], in_=fcc_ext[:])
            fcd_sb = cpool.tile([128, NSC, 64], F32)
            nc.sync.dma_start(out=fcd_sb[:], in_=fcd_ext[:])
            bias_sb = cpool.tile([128, DQ + DK + DK], F32)
            nc.sync.dma_start(out=bias_sb[:], in_=bqkv_ext[:])

            # persistent activations
            qT_all = ppool.tile([128, NH * S], BF16)    # [d, h*S + s]
            kT_all = ppool.tile([128, NKV * S], BF16)   # [d, g*S + s]
            v_all = ppool.tile([128, NSC * DK], BF16)   # [s2 in chunk, sc*DK + d]
            oT_all = ppool.tile([128, NH * S], BF16)    # [d, h*S + s]

            # ---------------- Phase 1: QKV + rope + transpose ----------------
            with (
                tc.tile_pool(name="p1sb", bufs=1) as p1,
                tc.tile_pool(name="p1ps", bufs=1, space="PSUM") as ps1,
            ):
                for sc in range(NSC):
                    # q/k/v accumulation psums: 4x q(512), k(512), v(512)
                    accs = [
                        ps1.tile([128, 512], F32, tag="acc", bufs=6, name=f"acc{sc}_{b}")
                        for b in range(6)
                    ]
                    for dc in range(NDC):
                        xt_t = p1.tile([128, S], BF16, tag="xt", bufs=3, name=f"xt{sc}_{dc}")
                        nc.sync.dma_start(out=xt_t[:], in_=xt_ext[dc])
                        wq_t = p1.tile([128, DQ], BF16, tag="wq", bufs=3, name=f"wq{sc}_{dc}")
                        nc.sync.dma_start(out=wq_t[:], in_=wq_ext[dc])
                        wkv_t = p1.tile([128, DK + DK], BF16, tag="wkv", bufs=3, name=f"wkv{sc}_{dc}")
                        nc.sync.dma_start(out=wkv_t[:], in_=wkv_ext[dc])

                        lhs = xt_t[:, sc * 128:(sc + 1) * 128]
                        st = dc == 0
                        sp = dc == NDC - 1
                        for b in range(4):
                            nc.tensor.matmul(
                                accs[b][:], lhs, wq_t[:, b * 512:(b + 1) * 512],
                                start=st, stop=sp)
                        nc.tensor.matmul(accs[4][:], lhs, wkv_t[:, 0:512],
                                         start=st, stop=sp)
                        nc.tensor.matmul(accs[5][:], lhs, wkv_t[:, 512:1024],
                                         start=st, stop=sp)

                    # evict with bias: q -> f32 (rope), k -> f32, v -> bf16
                    q_sb = p1.tile([128, DQ], F32, tag="q_sb", bufs=2, name=f"q_sb{sc}")
                    for b in range(4):
                        nc.vector.tensor_tensor(
                            out=q_sb[:, b * 512:(b + 1) * 512], in0=accs[b][:],
                            in1=bias_sb[:, b * 512:(b + 1) * 512],
                            op=mybir.AluOpType.add)
                    k_sb = p1.tile([128, DK], F32, tag="k_sb", bufs=2, name=f"k_sb{sc}")
                    nc.vector.tensor_tensor(
                        out=k_sb[:], in0=accs[4][:],
                        in1=bias_sb[:, DQ:DQ + DK], op=mybir.AluOpType.add)
                    nc.vector.tensor_tensor(
                        out=v_all[:, sc * DK:(sc + 1) * DK], in0=accs[5][:],
                        in1=bias_sb[:, DQ + DK:], op=mybir.AluOpType.add)

                    # rope on q: view [128, NH, 128], evens-first layout
                    for (t_sb, nh, coff) in ((q_sb, NH, 0), (k_sb, NKV, 0)):
                        t3 = t_sb.rearrange("p (h c) -> p h c", c=128)
                        E = t3[:, :, 0:64]
                        O = t3[:, :, 64:128]
                        Cb = _bcast_like(fcc_sb[:, sc:sc + 1, :], E)
                        Db = _bcast_like(fcd_sb[:, sc:sc + 1, :], E)
                        tmps = []
                        for idx, (a, bb) in enumerate(((E, Cb), (O, Db), (E, Db), (O, Cb))):
                            t = p1.tile([128, nh, 64], F32, tag=f"rt{idx}", bufs=2,
                                        name=f"rt{idx}_{sc}_{nh}")
                            nc.vector.tensor_tensor(out=t[:], in0=a, in1=bb,
                                                    op=mybir.AluOpType.mult)
                            tmps.append(t)
                        nc.vector.tensor_tensor(out=E, in0=tmps[0][:], in1=tmps[1][:],
                                                op=mybir.AluOpType.subtract)
                        nc.vector.tensor_tensor(out=O, in0=tmps[2][:], in1=tmps[3][:],
                                                op=mybir.AluOpType.add)

                    # transpose to qT/kT (PE), evict to bf16
                    for h in range(NH):
                        tp = ps1.tile([128, 128], F32, tag="tp", bufs=2, name=f"tpq{sc}_{h}")
                        nc.tensor.transpose(tp[:], q_sb[:, h * 128:(h + 1) * 128], ident_f)
                        nc.scalar.copy(
                            qT_all[:, h * S + sc * 128: h * S + (sc + 1) * 128], tp[:])
                    for g in range(NKV):
                        tp = ps1.tile([128, 128], F32, tag="tp", bufs=2, name=f"tpk{sc}_{g}")
                        nc.tensor.transpose(tp[:], k_sb[:, g * 128:(g + 1) * 128], ident_f)
                        nc.scalar.copy(
                            kT_all[:, g * S + sc * 128: g * S + (sc + 1) * 128], tp[:])

            # ---------------- Phase 2: attention ----------------
            with (
                tc.tile_pool(name="p2sb", bufs=1) as p2,
                tc.tile_pool(name="p2ps", bufs=1, space="PSUM") as ps2,
            ):
                for h in range(NH):
                    g = h // 4
                    denom = spool.tile([128, NSC], F32, tag="den", name=f"den{h}")
                    recip = spool.tile([128, NSC], F32, tag="rec", name=f"rec{h}")
                    attn = []
                    for i in range(NSC):
                        L = (i + 1) * 128
                        scp = ps2.tile([128, 1024], F32, tag="sc", bufs=2, name=f"scp{h}_{i}")
                        lhs = qT_all[:, h * S + i * 128: h * S + (i + 1) * 128]
                        for c0 in range(0, L, 512):
                            c1 = min(c0 + 512, L)
                            nc.tensor.matmul(
                                scp[:, c0:c1], lhs, kT_all[:, g * S + c0: g * S + c1],
                                start=True, stop=True)
                        # causal mask on diagonal block (in-place psum add)
                        nc.vector.tensor_tensor(
                            out=scp[:, i * 128:L], in0=scp[:, i * 128:L],
                            in1=maskbias[:], op=mybir.AluOpType.add)
                        a_i = p2.tile([128, 1024], BF16, tag=f"attn{i}", bufs=2,
                                      name=f"attn{h}_{i}")
                        nc.scalar.activation(
                            a_i[:, 0:L], scp[:, 0:L],
                            mybir.ActivationFunctionType.Exp,
                            scale=QK_SCALE, accum_out=denom[:, i:i + 1])
                        nc.vector.reciprocal(recip[:, i:i + 1], denom[:, i:i + 1])
                        nc.vector.tensor_scalar(
                            out=a_i[:, 0:L], in0=a_i[:, 0:L],
                            scalar1=recip[:, i:i + 1], scalar2=None,
                            op0=mybir.AluOpType.mult)
                        attn.append(a_i)

                    otp = ps2.tile([128, 1024], F32, tag="ot", bufs=1, name=f"otp{h}")
                    for j in range(NSC):
                        trp = ps2.tile([128, 1024], BF16, tag="trp", bufs=1, name=f"trp{h}_{j}")
                        for i in range(j, NSC):
                            nc.tensor.transpose(
                                trp[:, i * 128:(i + 1) * 128],
                                attn[i][:, j * 128:(j + 1) * 128], ident_b)
                        aT = p2.tile([128, 1024], BF16, tag="aT", bufs=3, name=f"aT{h}_{j}")
                        nc.scalar.copy(aT[:, j * 128:1024], trp[:, j * 128:1024])
                        vs = v_all[:, j * DK + g * 128: j * DK + (g + 1) * 128]
                        if j < 4:
                            chunks = [(j * 128, 512), (512, 1024)]
                        else:
                            chunks = [(j * 128, 1024)]
                        for (c0, c1) in chunks:
                            nc.tensor.matmul(
                                otp[:, c0:c1], vs, aT[:, c0:c1],
                                start=(j == 0), stop=(j == NSC - 1),
                                skip_group_check=True)
                    nc.scalar.copy(oT_all[:, h * S:(h + 1) * S], otp[:])

            # ---------------- Phase 3: Wo ----------------
            with (
                tc.tile_pool(name="p3sb", bufs=1) as p3,
                tc.tile_pool(name="p3ps", bufs=1, space="PSUM") as ps3,
            ):
                for sh in range(2):       # s halves
                    for mg in range(4):   # groups of 8 Dout-chunks
                        wps = [
                            ps3.tile([128, 512], F32, tag="wps", bufs=8,
                                     name=f"wps{sh}_{mg}_{m}")
                            for m in range(8)
                        ]
                        for c in range(NH):
                            wo_t = p3.tile([128, 1024], BF16, tag="wo", bufs=3,
                                           name=f"wo{sh}_{mg}_{c}")
                            nc.sync.dma_start(
                                out=wo_t[:],
                                in_=wo_ext[c, :, mg * 1024:(mg + 1) * 1024])
                            rhs = oT_all[:, c * S + sh * 512: c * S + sh * 512 + 512]
                            for m in range(8):
                                nc.tensor.matmul(
                                    wps[m][:], wo_t[:, m * 128:(m + 1) * 128], rhs,
                                    start=(c == 0), stop=(c == NH - 1))
                        for m in range(8):
                            ot_sb = p3.tile([128, 512], F32, tag="ot_sb", bufs=4,
                                            name=f"osb{sh}_{mg}_{m}")
                            nc.vector.tensor_copy(ot_sb[:], wps[m][:])
                            mm = mg * 8 + m
                            nc.sync.dma_start(
                                out=out_ext[mm * 128:(mm + 1) * 128,
                                            sh * 512:(sh + 1) * 512],
                                in_=ot_sb[:])

    nc.compile()
    return nc


def _evens_first_perm(nheads):
    idx = []
    for h in range(nheads):
        base = h * HD
        idx.extend(range(base, base + HD, 2))
        idx.extend(range(base + 1, base + HD, 2))
    return np.array(idx, dtype=np.int64)


def kernel(x, freqs_cis, Wq, bq, Wk, bk, Wv, bv, Wo, bo, startpos):
    global LAST_PROFILE
    x = np.asarray(x, dtype=np.float32)
    freqs_cis = np.asarray(freqs_cis, dtype=np.float32)
    Wq = np.asarray(Wq, dtype=np.float32)
    Wk = np.asarray(Wk, dtype=np.float32)
    Wv = np.asarray(Wv, dtype=np.float32)
    Wo = np.asarray(Wo, dtype=np.float32)
    bq = np.asarray(bq, dtype=np.float32)
    bk = np.asarray(bk, dtype=np.float32)
    bv = np.asarray(bv, dtype=np.float32)
    bo = np.asarray(bo, dtype=np.float32)
    assert int(startpos) == 0

    bf = lambda a: np.ascontiguousarray(a.astype(ml_dtypes.bfloat16))
    f32c = lambda a: np.ascontiguousarray(a.astype(np.float32))

    fcc = f32c(freqs_cis[:, :, 0].reshape(NSC, 128, 64).transpose(1, 0, 2))
    fcd = f32c(freqs_cis[:, :, 1].reshape(NSC, 128, 64).transpose(1, 0, 2))

    in_maps = []
    for core in range(8):
        b, g = core // 2, core % 2
        qsel = g * DQ + _evens_first_perm(NH)
        ksel = g * DK + _evens_first_perm(NKV)
        vsel = np.arange(g * DK, (g + 1) * DK)
        wq_h = bf(Wq[:, qsel].reshape(NDC, 128, DQ))
        wkv_h = bf(np.concatenate([Wk[:, ksel], Wv[:, vsel]], 1).reshape(NDC, 128, DK + DK))
        wo_h = bf(Wo[g * DQ:(g + 1) * DQ, :].reshape(NH, 128, D))
        xt_h = bf(x[b].T.reshape(NDC, 128, S))
        bqkv = np.concatenate([bq[qsel], bk[ksel], bv[vsel]])
        bqkv = f32c(np.tile(bqkv[None, :], (128, 1)))
        in_maps.append({
            "xt": xt_h, "wq": wq_h, "wkv": wkv_h, "wo": wo_h,
            "fcc": fcc, "fcd": fcd, "bqkv": bqkv,
        })

    if "nc" not in _GRAPH_CACHE:
        _GRAPH_CACHE["nc"] = _build_graph()
    nc = _GRAPH_CACHE["nc"]

    res = run_bass_kernel_spmd(
        nc, in_maps, core_ids=list(range(8)),
        trace=bool(os.environ.get("BASS_TRACE")))
    LAST_PROFILE = res

    out = np.empty((B, S, D), dtype=np.float32)
    for b in range(B):
        t = res.results[2 * b]["out"] + res.results[2 * b + 1]["out"]
        out[b] = t.T + bo[None, :]
    return out
